# revision 1
# baseline (speedup 1.0000x reference)
"""Fused OT-DTW l2 cost-matrix kernel for Trainium2 (8 NeuronCores, SPMD).

mat_cost[i,j] = sum_{t,p,d} pi[cl(i)][t,p] * (X[i,t,d] - Y[j,p,d])^2
             = C1[i] + C2[cl(i), j] - 2 * C3[i,j]

with C3[i,j] = sum_{p,d} XP[i,p,d] * Y[j,p,d],  XP[i] = X[i].T @ pi[cl(i)]
(as [D,TP], i.e. XPT). The device computes the heavy parts (XP: ~69 GFLOP,
C3: ~137 GFLOP) in fp8e4m3 on 8 cores, data-parallel over rows of X (pi is
0/1 so fp8 is exact for it; X/Y quantization error washes out over the
65536-term contraction: measured ~7e-4 of output absmax). The tiny rank-1
corrections C1/C2 (<0.2% of FLOPs) are applied on the host in fp32.

Sharding: core k takes X rows [128k, 128k+128), Y/pi replicated. Host
pre-permutes operands so every device DMA is contiguous:
  xs   [i, t, c, d]    (c = t-chunk of 128) fp8, per core (8.4MB)
  pi_d [t, c, cls*p]   all 8 classes, fp8, resident in SBUF (2MB)
  offs [1, i]          classe*TP int32 -> per-row dynamic rhs offset
  yt   [d, p, j]       Y transposed, fp8, streamed (67MB; the roofline term)
Per core: a short scratch-matmul burst first warms the PE clock-gate. Stage
A computes XPT_i[d,p] = X[i].T @ pi_cl(i) (4 accumulating matmuls, K=t
tiles of 128, N=512; class picked via register-offset AP into resident pi),
then corner-turns PSUM->SBUF into xpt[d, p, i] fp8 with packed 4-row casts
split across DVE/ACT. Stage B accumulates C3[i, j] over 256 DoubleRow
p-pairs: lhsT = xpt[:, p:p+2, :], rhs = yt tile [d, 8p, 1024j] streamed at
~420 GB/s with a 12-buffer ring prefilled during stage A.
"""

import os
import sys
import types

import numpy as np
import ml_dtypes

NX, NY, T, TP, D, C = 1024, 1024, 512, 512, 128, 8
N_CORES = 8
NL = NX // N_CORES          # 128 rows per core
TC = T // 128               # 4 t-chunks
PG = 8                      # p-tiles per Y DMA
BF16 = ml_dtypes.bfloat16
F8 = ml_dtypes.float8_e4m3fn


def _ensure_axon_hooks():
    """concourse.bass_utils imports antenv.axon_hooks when tracing under
    axon; some images lack that submodule. Provide it, and register the
    NTFF profile hook if the boot path didn't."""
    try:
        import antenv
    except ImportError:
        return
    try:
        from antenv import axon_hooks  # noqa: F401
    except ImportError:
        mod = types.ModuleType("antenv.axon_hooks")
        mod._hook = None

        def _set(h):
            mod._hook = h

        def _get():
            return mod._hook

        mod.set_axon_ntff_profile_hook = _set
        mod.get_axon_ntff_profile_hook = _get
        sys.modules["antenv.axon_hooks"] = mod
        antenv.axon_hooks = mod
    from antenv.axon_hooks import (
        get_axon_ntff_profile_hook,
        set_axon_ntff_profile_hook,
    )

    if get_axon_ntff_profile_hook() is None:
        try:
            from trn_agent_boot.trn_boot import _ntff_profile_via_ctypes

            hook = _ntff_profile_via_ctypes("/opt/axon/libaxon_pjrt.so")
            if hook is not None:
                set_axon_ntff_profile_hook(hook)
        except Exception:
            pass


_ensure_axon_hooks()

import concourse.bass as bass  # noqa: E402
import concourse.tile as tile  # noqa: E402
from concourse import bacc, mybir  # noqa: E402
from concourse.bass_utils import run_bass_kernel_spmd  # noqa: E402

_PROGRAM_CACHE = {}
LAST_RUN = None  # BassKernelResults of the most recent kernel() call


def _build_program():
    if "nc" in _PROGRAM_CACHE:
        return _PROGRAM_CACHE["nc"]
    f8 = mybir.dt.float8e4
    f32 = mybir.dt.float32
    i32 = mybir.dt.int32
    DR = mybir.MatmulPerfMode.DoubleRow
    nc = bacc.Bacc("TRN2", target_bir_lowering=False, debug=False,
                   num_devices=N_CORES)
    xs = nc.dram_tensor("xs", [NL, 128, TC, D], f8, kind="ExternalInput").ap()
    pi_d = nc.dram_tensor("pi_d", [128, TC, C * TP], f8, kind="ExternalInput").ap()
    offs = nc.dram_tensor("offs", [1, NL], i32, kind="ExternalInput").ap()
    yt = nc.dram_tensor("yt", [D, TP, NY], f8, kind="ExternalInput").ap()
    c3 = nc.dram_tensor("c3", [NL, NY], f32, kind="ExternalOutput").ap()

    with tile.TileContext(nc) as tc:
        with (
            tc.tile_pool(name="xpt", bufs=1) as xpt_pool,
            tc.tile_pool(name="xin", bufs=4) as xin_pool,
            tc.tile_pool(name="pisb", bufs=1) as pi_pool,
            tc.tile_pool(name="yin", bufs=12) as y_pool,
            tc.tile_pool(name="outsb", bufs=1) as out_pool,
        ):
            # Resident transposed XP for all local rows: [d, p, i] fp8
            # (p-major pairs for DoubleRow lhsT interleave).
            xpt = xpt_pool.tile([D, TP, NL], f8)

            # PE warmup: ~20 matmuls on scratch data at t=0, overlapping the
            # first input DMAs, so the HAM clock-gate reaches 8/8 before the
            # real matmuls start (values never read; NaNs harmless).
            with (
                tc.tile_pool(name="warm", bufs=1) as warm_pool,
                tc.tile_pool(name="warmps", bufs=1, space="PSUM") as warmps_pool,
            ):
                wsrc = warm_pool.tile([128, 512], f8)
                wacc = warmps_pool.tile([128, 512], f32)
                nc.gpsimd.memset(wsrc[:], 0.0)
                for w in range(14):
                    nc.tensor.matmul(wacc[:], wsrc[:, 0:128], wsrc[:],
                                     start=True, stop=True)

            # ---- Stage A: XPT_i = X[i].T @ pi[cl(i)] ----
            # pi resident in SBUF (all classes, 2MB); per-row class selected
            # via register offset read from `offs` (host = classe * TP).
            off_sb = pi_pool.tile([1, NL], i32)
            nc.sync.dma_start(off_sb[:], offs[:])
            pi_sb = pi_pool.tile([128, TC, C * TP], f8)
            for c in range(TC):   # per-chunk loads: first matmul only waits c=0
                nc.sync.dma_start(pi_sb[:, c, :], pi_d[:, c, :])
            XB, CB = 4, 4   # xs rows per DMA, rows per packed cast
            with tc.tile_pool(name="psA", bufs=2, space="PSUM") as psA_pool:
                for i in range(NL):
                    if i % XB == 0:
                        xt = xin_pool.tile([128, XB, TC, D], f8, tag="xt")
                        nc.sync.dma_start(
                            xt[:], xs[i:i + XB].rearrange("a t c d -> t a c d"))
                    if i % CB == 0:
                        acc = psA_pool.tile([D, CB, TP], f32)  # 4 PSUM banks
                    xv = xt[:, i % XB]
                    off = nc.values_load(
                        off_sb[0:1, i:i + 1], engines=[mybir.EngineType.PE],
                        min_val=0, max_val=(C - 1) * TP,
                        skip_runtime_bounds_check=True)
                    for c in range(TC):
                        nc.tensor.matmul(
                            acc[:, i % CB, :],
                            xv[:, c, :], pi_sb[:, c, bass.ds(off, TP)],
                            start=(c == 0), stop=(c == TC - 1),
                        )
                    if i % CB == CB - 1:
                        # Packed corner-turn: psum[d, 4, p] -> xpt[d, p, i..i+3]
                        # (4B inner runs @128B stride; 4x the 1B-run rate).
                        # Split p-range across DVE and ACT to halve latency.
                        g0 = i - (CB - 1)
                        h = TP // 2
                        s0 = acc[:, :, 0:h].rearrange("d k p -> d p k")
                        s1 = acc[:, :, h:TP].rearrange("d k p -> d p k")
                        nc.vector.tensor_copy(xpt[:, 0:h, g0:g0 + CB], s0)
                        nc.scalar.copy(xpt[:, h:TP, g0:g0 + CB], s1)

            # ---- Stage B: C3[i, j] = sum_p XPT[:, p, i] YT[:, p, j], DR pairs ----
            with tc.tile_pool(name="psB", bufs=1, space="PSUM") as psB_pool:
                accj0 = psB_pool.tile([NL, 512], f32)
                accj1 = psB_pool.tile([NL, 512], f32)
                for g in range(TP // PG):
                    ytile = y_pool.tile([D, PG, NY], f8)
                    nc.sync.dma_start(ytile[:], yt[:, g * PG:(g + 1) * PG, :])
                    for s in range(PG // 2):
                        p = g * PG + 2 * s
                        st, sp = (p == 0), (p == TP - 2)
                        lhsT = xpt[:, p:p + 2, :]
                        rhs = ytile[:, 2 * s:2 * s + 2, :]
                        nc.tensor.matmul(accj0[:], lhsT, rhs[:, :, 0:512],
                                         start=st, stop=sp, perf_mode=DR)
                        nc.tensor.matmul(accj1[:], lhsT, rhs[:, :, 512:1024],
                                         start=st, stop=sp, perf_mode=DR)

            out_sb = out_pool.tile([NL, NY], f32)
            nc.vector.tensor_copy(out_sb[:, 0:512], accj0[:])
            nc.scalar.copy(out_sb[:, 512:1024], accj1[:])
            nc.sync.dma_start(c3[:], out_sb[:])

    nc.compile()
    _PROGRAM_CACHE["nc"] = nc
    return nc


def kernel(X, Y, pi, classe):
    global LAST_RUN
    assert X.shape == (NX, T, D) and Y.shape == (NY, TP, D)
    assert pi.shape == (C, T, TP) and classe.shape == (NX,)
    X = np.asarray(X, dtype=np.float32)
    Y = np.asarray(Y, dtype=np.float32)
    pi = np.asarray(pi, dtype=np.float32)
    classe = np.asarray(classe)

    nc = _build_program()

    # Host-side sharding + layout prep (all-contiguous device DMAs).
    # pi_d[t, c, cls, p] = pi[cls, c*128 + t, p], all classes resident.
    pi_d = np.ascontiguousarray(
        pi.astype(F8).reshape(C, TC, 128, TP).transpose(2, 1, 0, 3)
    ).reshape(128, TC, C * TP)
    # yt[d, p, j] = Y[j, p, d]
    yt = np.ascontiguousarray(Y.transpose(2, 1, 0).astype(F8))
    in_maps = []
    for k in range(N_CORES):
        rows = slice(k * NL, (k + 1) * NL)
        xk = X[rows].astype(F8)                        # [NL, T, D]
        xk = np.ascontiguousarray(
            xk.reshape(NL, TC, 128, D).transpose(0, 2, 1, 3))
        offs = (classe[rows].astype(np.int32) * TP)[None, :]
        in_maps.append({"xs": xk, "pi_d": pi_d, "offs": offs, "yt": yt})

    trace = bool(os.environ.get("BASS_TRACE"))
    LAST_RUN = run_bass_kernel_spmd(nc, in_maps, list(range(N_CORES)),
                                    trace=trace)
    C3 = np.concatenate([LAST_RUN.results[k]["c3"] for k in range(N_CORES)],
                        axis=0)

    # Host epilogue: rank-1 corrections (0.15% of FLOPs).
    row_c = pi.sum(-1)                                 # [C, T]
    col_c = pi.sum(1)                                  # [C, TP]
    SX = np.einsum("itd,itd->it", X, X)                # [NX, T]
    SY = np.einsum("jpd,jpd->jp", Y, Y)                # [NY, TP]
    C1 = np.einsum("it,it->i", SX, row_c[classe])      # [NX]
    C2 = col_c @ SY.T                                  # [C, NY]
    return (C1[:, None] + C2[classe] - 2.0 * C3).astype(np.float32)



# revision 2
# speedup vs baseline: 1.8274x; 1.8274x over previous
"""Fused OT-DTW l2 cost-matrix kernel for Trainium2 (8 NeuronCores, SPMD).

mat_cost[i,j] = sum_{t,p,d} pi[cl(i)][t,p] * (X[i,t,d] - Y[j,p,d])^2
             = C1[i] + C2[cl(i), j] - 2 * C3[i,j]

with C3[i,j] = sum_{p,d} XP[i,p,d] * Y[j,p,d],  XP[i] = X[i].T @ pi[cl(i)].

Key structural fact: pi[c] is a monotone DTW *path* matrix, so each column p
covers a contiguous t-interval [t0(p), t1(p)]. Hence
    XP[i,p,:] = SX[i, t1(p)+1, :] - SX[i, t0(p), :],   SX = cumsum_t(X[i])
— the 69-GFLOP "stage A" collapses to a prefix-sum + gather-diff, done
exactly in f32 on the host (~100 MFLOP). The device runs only the heavy
C3 contraction (137 GFLOP) in fp8 DoubleRow.

Sharding: 4x2 grid — core k=(a,b): rows [256a, 256a+256) x cols
[512b, 512b+512). Per-core HBM traffic: xpt 16.8MB + yt 33.5MB (vs 75.5MB
for 1D row sharding) against a ~430 GB/s per-core DMA ceiling. Host
pre-permutes so every DMA is contiguous:
  xpt [d, p, i]   fp8, chunk-loaded, resident in SBUF (128KB/partition)
  yt  [d, p, j]   fp8, streamed in 1MB tiles through an 8-buffer ring
Per core: short scratch-matmul warmup for the PE clock-gate, then 512
DoubleRow matmuls (p-pairs; lhsT = xpt[:, p:p+2, iblock], rhs = ytile)
accumulating C3 into 2 PSUM banks. The tiny rank-1 corrections C1/C2
(<0.2% of FLOPs) are applied on the host in fp32.
"""

import os
import sys
import types

import numpy as np
import ml_dtypes

NX, NY, T, TP, D, C = 1024, 1024, 512, 512, 128, 8
N_CORES = 8
GA, GB = 4, 2               # core grid: 4 i-groups x 2 j-groups
NLI = NX // GA              # 256 rows per core
NJ = NY // GB               # 512 cols per core
PG = 16                     # p-values per Y DMA tile (1MB)
XCH = 8                     # xpt chunk loads (2.1MB each)
F8 = ml_dtypes.float8_e4m3fn


def _ensure_axon_hooks():
    """concourse.bass_utils imports antenv.axon_hooks when tracing under
    axon; some images lack that submodule. Provide it, and register the
    NTFF profile hook if the boot path didn't."""
    try:
        import antenv
    except ImportError:
        return
    try:
        from antenv import axon_hooks  # noqa: F401
    except ImportError:
        mod = types.ModuleType("antenv.axon_hooks")
        mod._hook = None

        def _set(h):
            mod._hook = h

        def _get():
            return mod._hook

        mod.set_axon_ntff_profile_hook = _set
        mod.get_axon_ntff_profile_hook = _get
        sys.modules["antenv.axon_hooks"] = mod
        antenv.axon_hooks = mod
    from antenv.axon_hooks import (
        get_axon_ntff_profile_hook,
        set_axon_ntff_profile_hook,
    )

    if get_axon_ntff_profile_hook() is None:
        try:
            from trn_agent_boot.trn_boot import _ntff_profile_via_ctypes

            hook = _ntff_profile_via_ctypes("/opt/axon/libaxon_pjrt.so")
            if hook is not None:
                set_axon_ntff_profile_hook(hook)
        except Exception:
            pass


_ensure_axon_hooks()

import concourse.bass as bass  # noqa: E402, F401
import concourse.tile as tile  # noqa: E402
from concourse import bacc, mybir  # noqa: E402
from concourse.bass_utils import run_bass_kernel_spmd  # noqa: E402

_PROGRAM_CACHE = {}
LAST_RUN = None  # BassKernelResults of the most recent kernel() call


def _build_program():
    if "nc" in _PROGRAM_CACHE:
        return _PROGRAM_CACHE["nc"]
    f8 = mybir.dt.float8e4
    f32 = mybir.dt.float32
    DR = mybir.MatmulPerfMode.DoubleRow
    nc = bacc.Bacc("TRN2", target_bir_lowering=False, debug=False,
                   num_devices=N_CORES)
    xpt = nc.dram_tensor("xpt", [D, TP, NLI], f8, kind="ExternalInput").ap()
    yt = nc.dram_tensor("yt", [D, TP, NJ], f8, kind="ExternalInput").ap()
    c3 = nc.dram_tensor("c3", [NLI, NJ], f32, kind="ExternalOutput").ap()

    with tile.TileContext(nc) as tc:
        with (
            tc.tile_pool(name="xptsb", bufs=1) as xpt_pool,
            tc.tile_pool(name="yin", bufs=8) as y_pool,
            tc.tile_pool(name="outsb", bufs=1) as out_pool,
        ):
            xpt_sb = xpt_pool.tile([D, TP, NLI], f8)

            # PE warmup: scratch matmuls at t=0, overlapping the first input
            # DMAs, so the HAM clock-gate reaches full speed before the real
            # matmuls start (values never read).
            with (
                tc.tile_pool(name="warm", bufs=1) as warm_pool,
                tc.tile_pool(name="warmps", bufs=1, space="PSUM") as warmps_pool,
            ):
                wsrc = warm_pool.tile([128, 512], f8)
                wacc = warmps_pool.tile([128, 512], f32)
                nc.gpsimd.memset(wsrc[:], 0.0)
                for w in range(14):
                    nc.tensor.matmul(wacc[:], wsrc[:, 0:128], wsrc[:],
                                     start=True, stop=True)

            # Interleave xpt chunk loads with the yt stream so the first
            # matmul only waits on chunk 0 + tile 0, and chunk c arrives
            # well before the PE reaches p = 64c.
            pc = TP // XCH  # p-values per xpt chunk

            with tc.tile_pool(name="ps", bufs=1, space="PSUM") as ps_pool:
                acc0 = ps_pool.tile([128, NJ], f32)
                acc1 = ps_pool.tile([128, NJ], f32)
                for g in range(TP // PG):
                    if g < 2 * XCH and g % 2 == 0:
                        ch = g // 2
                        nc.sync.dma_start(
                            xpt_sb[:, ch * pc:(ch + 1) * pc, :],
                            xpt[:, ch * pc:(ch + 1) * pc, :])
                    ytile = y_pool.tile([D, PG, NJ], f8)
                    nc.sync.dma_start(ytile[:], yt[:, g * PG:(g + 1) * PG, :])
                    for s in range(PG // 2):
                        p = g * PG + 2 * s
                        st, sp = (p == 0), (p == TP - 2)
                        rhs = ytile[:, 2 * s:2 * s + 2, :]
                        nc.tensor.matmul(acc0[:], xpt_sb[:, p:p + 2, 0:128],
                                         rhs, start=st, stop=sp, perf_mode=DR)
                        nc.tensor.matmul(acc1[:], xpt_sb[:, p:p + 2, 128:256],
                                         rhs, start=st, stop=sp, perf_mode=DR)

            out_sb = out_pool.tile([128, 2, NJ], f32)
            nc.vector.tensor_copy(out_sb[:, 0, :], acc0[:])
            nc.scalar.copy(out_sb[:, 1, :], acc1[:])
            nc.sync.dma_start(c3[0:128, :], out_sb[:, 0, :])
            nc.sync.dma_start(c3[128:NLI, :], out_sb[:, 1, :])

    nc.compile()
    _PROGRAM_CACHE["nc"] = nc
    return nc


def _host_xp(X, pi, classe):
    """XP[i,p,:] = sum_t pi[cl(i)][t,p] * X[i,t,:], exact f32.

    Fast path uses the DTW-path structure (each pi column covers a
    contiguous t-interval): prefix sums + gather-diff. Falls back to
    per-class BLAS if any column is non-contiguous or empty."""
    nz = pi > 0.5                                    # [C, T, TP]
    cnt = nz.sum(axis=1)                             # [C, TP]
    tidx = np.arange(T, dtype=np.int64)[None, :, None]
    t1 = np.where(nz, tidx, -1).max(axis=1)          # [C, TP]
    t0 = np.where(nz, tidx, T).min(axis=1)           # [C, TP]
    if (cnt > 0).all() and (cnt == t1 - t0 + 1).all():
        SX = np.zeros((NX, T + 1, D), dtype=np.float32)
        np.cumsum(X, axis=1, out=SX[:, 1:, :])
        ar = np.arange(NX)[:, None]
        return SX[ar, t1[classe] + 1, :] - SX[ar, t0[classe], :]
    XP = np.empty((NX, TP, D), dtype=np.float32)
    for c in range(C):
        m = classe == c
        if m.any():
            # [n,D,T] @ [T,TP] -> [n,D,TP] -> [n,TP,D]
            XP[m] = np.matmul(X[m].transpose(0, 2, 1), pi[c]).transpose(0, 2, 1)
    return XP


def kernel(X, Y, pi, classe):
    global LAST_RUN
    assert X.shape == (NX, T, D) and Y.shape == (NY, TP, D)
    assert pi.shape == (C, T, TP) and classe.shape == (NX,)
    X = np.asarray(X, dtype=np.float32)
    Y = np.asarray(Y, dtype=np.float32)
    pi = np.asarray(pi, dtype=np.float32)
    classe = np.asarray(classe)

    nc = _build_program()

    XP8 = _host_xp(X, pi, classe).astype(F8)         # [NX, TP, D]
    Y8 = Y.astype(F8)                                # [NY, TP, D]
    xpts = [np.ascontiguousarray(
        XP8[a * NLI:(a + 1) * NLI].transpose(2, 1, 0)) for a in range(GA)]
    yts = [np.ascontiguousarray(
        Y8[b * NJ:(b + 1) * NJ].transpose(2, 1, 0)) for b in range(GB)]
    in_maps = [{"xpt": xpts[k // GB], "yt": yts[k % GB]}
               for k in range(N_CORES)]

    trace = bool(os.environ.get("BASS_TRACE"))
    LAST_RUN = run_bass_kernel_spmd(nc, in_maps, list(range(N_CORES)),
                                    trace=trace)
    C3 = np.empty((NX, NY), dtype=np.float32)
    for k in range(N_CORES):
        a, b = k // GB, k % GB
        C3[a * NLI:(a + 1) * NLI, b * NJ:(b + 1) * NJ] = \
            LAST_RUN.results[k]["c3"]

    # Host epilogue: rank-1 corrections (0.15% of FLOPs).
    row_c = pi.sum(-1)                                 # [C, T]
    col_c = pi.sum(1)                                  # [C, TP]
    SX = np.einsum("itd,itd->it", X, X)                # [NX, T]
    SY = np.einsum("jpd,jpd->jp", Y, Y)                # [NY, TP]
    C1 = np.einsum("it,it->i", SX, row_c[classe])      # [NX]
    C2 = col_c @ SY.T                                  # [C, NY]
    return (C1[:, None] + C2[classe] - 2.0 * C3).astype(np.float32)


# revision 5
# speedup vs baseline: 1.9310x; 1.0567x over previous
"""Fused OT-DTW l2 cost-matrix kernel for Trainium2 (8 NeuronCores, SPMD).

mat_cost[i,j] = sum_{t,p,d} pi[cl(i)][t,p] * (X[i,t,d] - Y[j,p,d])^2
             = C1[i] + C2[cl(i), j] - 2 * C3[i,j]

with C3[i,j] = sum_{p,d} XP[i,p,d] * Y[j,p,d],  XP[i] = X[i].T @ pi[cl(i)].

Key structural fact: pi[c] is a monotone DTW *path* matrix, so each column p
covers a contiguous t-interval [t0(p), t1(p)]. Hence
    XP[i,p,:] = SX[i, t1(p)+1, :] - SX[i, t0(p), :],   SX = cumsum_t(X[i])
— the 69-GFLOP "stage A" collapses to a prefix-sum + gather-diff, done
exactly in f32 on the host (~100 MFLOP). The device runs only the heavy
C3 contraction (137 GFLOP) in fp8 DoubleRow.

Sharding: 4x2 grid — core k=(a,b): rows [256a, 256a+256) x cols
[512b, 512b+512). Per-core HBM traffic: xpt 16.8MB + yt 33.5MB (vs 75.5MB
for 1D row sharding) against a ~430 GB/s per-core DMA ceiling. Host
pre-permutes so every DMA is contiguous:
  xpt [d, p, i]   fp8, chunk-loaded, resident in SBUF (128KB/partition)
  yt  [d, p, j]   fp8, streamed in 1MB tiles through an 8-buffer ring
Per core: short scratch-matmul warmup for the PE clock-gate, then 512
DoubleRow matmuls (p-pairs; lhsT = xpt[:, p:p+2, iblock], rhs = ytile)
accumulating C3 into 2 PSUM banks. The tiny rank-1 corrections C1/C2
(<0.2% of FLOPs) are applied on the host in fp32.
"""

import os
import sys
import types

import numpy as np
import ml_dtypes

NX, NY, T, TP, D, C = 1024, 1024, 512, 512, 128, 8
N_CORES = 8
GA, GB = 4, 2               # core grid: 4 i-groups x 2 j-groups
NLI = NX // GA              # 256 rows per core
NJ = NY // GB               # 512 cols per core
PG = 16                     # p-values per Y DMA tile (1MB)
XCH = 8                     # xpt chunk loads (2.1MB each)
F8 = ml_dtypes.float8_e4m3fn


def _ensure_axon_hooks():
    """concourse.bass_utils imports antenv.axon_hooks when tracing under
    axon; some images lack that submodule. Provide it, and register the
    NTFF profile hook if the boot path didn't."""
    try:
        import antenv
    except ImportError:
        return
    try:
        from antenv import axon_hooks  # noqa: F401
    except ImportError:
        mod = types.ModuleType("antenv.axon_hooks")
        mod._hook = None

        def _set(h):
            mod._hook = h

        def _get():
            return mod._hook

        mod.set_axon_ntff_profile_hook = _set
        mod.get_axon_ntff_profile_hook = _get
        sys.modules["antenv.axon_hooks"] = mod
        antenv.axon_hooks = mod
    from antenv.axon_hooks import (
        get_axon_ntff_profile_hook,
        set_axon_ntff_profile_hook,
    )

    if get_axon_ntff_profile_hook() is None:
        try:
            from trn_agent_boot.trn_boot import _ntff_profile_via_ctypes

            hook = _ntff_profile_via_ctypes("/opt/axon/libaxon_pjrt.so")
            if hook is not None:
                set_axon_ntff_profile_hook(hook)
        except Exception:
            pass


_ensure_axon_hooks()

import concourse.bass as bass  # noqa: E402, F401
import concourse.tile as tile  # noqa: E402
from concourse import bacc, mybir  # noqa: E402
from concourse.bass_utils import run_bass_kernel_spmd  # noqa: E402

_PROGRAM_CACHE = {}
LAST_RUN = None  # BassKernelResults of the most recent kernel() call


SWI = True  # DoubleRowSwInterleave weights (host pre-interleaved layout)


def _build_program():
    if "nc" in _PROGRAM_CACHE:
        return _PROGRAM_CACHE["nc"]
    f8 = mybir.dt.float8e4
    f32 = mybir.dt.float32
    PM = (mybir.MatmulPerfMode.DoubleRowSwInterleave if SWI
          else mybir.MatmulPerfMode.DoubleRow)
    nc = bacc.Bacc("TRN2", target_bir_lowering=False, debug=False,
                   num_devices=N_CORES)
    # SWI layout: [d, p-pair, iblock, 256 interleaved-reversed weights];
    # plain DR: [d, p, i].  Same bytes per core either way (16.8MB).
    xshape = [D, TP // 2, 2, 256] if SWI else [D, TP, NLI]
    xpt = nc.dram_tensor("xpt", xshape, f8, kind="ExternalInput").ap()
    yt = nc.dram_tensor("yt", [D, TP, NJ], f8, kind="ExternalInput").ap()
    c3 = nc.dram_tensor("c3", [NLI, NJ], f32, kind="ExternalOutput").ap()

    with tile.TileContext(nc) as tc:
        with (
            tc.tile_pool(name="xptsb", bufs=1) as xpt_pool,
            tc.tile_pool(name="yin", bufs=8) as y_pool,
            tc.tile_pool(name="outsb", bufs=1) as out_pool,
        ):
            xpt_sb = xpt_pool.tile(xshape, f8)

            # PE warmup: scratch matmuls at t=0, overlapping the first input
            # DMAs, so the HAM clock-gate reaches full speed before the real
            # matmuls start (values never read).
            with (
                tc.tile_pool(name="warm", bufs=1) as warm_pool,
                tc.tile_pool(name="warmps", bufs=1, space="PSUM") as warmps_pool,
            ):
                wsrc = warm_pool.tile([128, 512], f8)
                wacc = warmps_pool.tile([128, 512], f32)
                nc.gpsimd.memset(wsrc[:], 0.0)
                for w in range(14):
                    nc.tensor.matmul(wacc[:], wsrc[:, 0:128], wsrc[:],
                                     start=True, stop=True)

            # Interleave xpt chunk loads (SWDGE/gpsimd — its own DMA path)
            # with the yt stream (alternating between the two HWDGE rings,
            # sync and scalar) so the first matmul only waits on chunk 0 +
            # tile 0, and chunk c arrives well before the PE needs it.
            pc = TP // XCH  # p-values per xpt chunk

            with tc.tile_pool(name="ps", bufs=1, space="PSUM") as ps_pool:
                acc0 = ps_pool.tile([128, NJ], f32)
                acc1 = ps_pool.tile([128, NJ], f32)
                for g in range(TP // PG):
                    if g < 2 * XCH and g % 2 == 0:
                        ch = g // 2
                        if SWI:
                            src = xpt[:, ch * (pc // 2):(ch + 1) * (pc // 2)]
                            dst = xpt_sb[:, ch * (pc // 2):(ch + 1) * (pc // 2)]
                        else:
                            src = xpt[:, ch * pc:(ch + 1) * pc, :]
                            dst = xpt_sb[:, ch * pc:(ch + 1) * pc, :]
                        nc.gpsimd.dma_start(dst, src)
                    ytile = y_pool.tile([D, PG, NJ], f8)
                    dma_eng = nc.sync if g % 2 == 0 else nc.scalar
                    dma_eng.dma_start(ytile[:], yt[:, g * PG:(g + 1) * PG, :])
                    for s in range(PG // 2):
                        p = g * PG + 2 * s
                        st, sp = (p == 0), (p == TP - 2)
                        rhs = ytile[:, 2 * s:2 * s + 2, :]
                        if SWI:
                            w0 = xpt_sb[:, p // 2, 0, :]
                            w1 = xpt_sb[:, p // 2, 1, :]
                        else:
                            w0 = xpt_sb[:, p:p + 2, 0:128]
                            w1 = xpt_sb[:, p:p + 2, 128:256]
                        nc.tensor.matmul(acc0[:], w0, rhs,
                                         start=st, stop=sp, perf_mode=PM)
                        nc.tensor.matmul(acc1[:], w1, rhs,
                                         start=st, stop=sp, perf_mode=PM)

            out_sb = out_pool.tile([128, 2, NJ], f32)
            nc.vector.tensor_copy(out_sb[:, 0, :], acc0[:])
            nc.scalar.copy(out_sb[:, 1, :], acc1[:])
            nc.sync.dma_start(c3[0:128, :], out_sb[:, 0, :])
            nc.sync.dma_start(c3[128:NLI, :], out_sb[:, 1, :])

    nc.compile()
    _PROGRAM_CACHE["nc"] = nc
    return nc


def _host_xp(X, pi, classe):
    """XP[i,p,:] = sum_t pi[cl(i)][t,p] * X[i,t,:], exact f32.

    Fast path uses the DTW-path structure (each pi column covers a
    contiguous t-interval): prefix sums + gather-diff. Falls back to
    per-class BLAS if any column is non-contiguous or empty."""
    nz = pi > 0.5                                    # [C, T, TP]
    cnt = nz.sum(axis=1)                             # [C, TP]
    tidx = np.arange(T, dtype=np.int64)[None, :, None]
    t1 = np.where(nz, tidx, -1).max(axis=1)          # [C, TP]
    t0 = np.where(nz, tidx, T).min(axis=1)           # [C, TP]
    if (cnt > 0).all() and (cnt == t1 - t0 + 1).all():
        SX = np.zeros((NX, T + 1, D), dtype=np.float32)
        np.cumsum(X, axis=1, out=SX[:, 1:, :])
        ar = np.arange(NX)[:, None]
        return SX[ar, t1[classe] + 1, :] - SX[ar, t0[classe], :]
    XP = np.empty((NX, TP, D), dtype=np.float32)
    for c in range(C):
        m = classe == c
        if m.any():
            # [n,D,T] @ [T,TP] -> [n,D,TP] -> [n,TP,D]
            XP[m] = np.matmul(X[m].transpose(0, 2, 1), pi[c]).transpose(0, 2, 1)
    return XP


def kernel(X, Y, pi, classe):
    global LAST_RUN
    assert X.shape == (NX, T, D) and Y.shape == (NY, TP, D)
    assert pi.shape == (C, T, TP) and classe.shape == (NX,)
    X = np.asarray(X, dtype=np.float32)
    Y = np.asarray(Y, dtype=np.float32)
    pi = np.asarray(pi, dtype=np.float32)
    classe = np.asarray(classe)

    nc = _build_program()

    XP8 = _host_xp(X, pi, classe).astype(F8)         # [NX, TP, D]
    Y8 = Y.astype(F8)                                # [NY, TP, D]
    xpts = []
    for a in range(GA):
        arr = XP8[a * NLI:(a + 1) * NLI].transpose(2, 1, 0)  # [d, p, i]
        if SWI:
            # Per (pair p2, iblock): HW-native weight stream
            # [A(127), B(127), ..., A(0), B(0)] — pairs interleaved,
            # columns reversed (A = p even, B = p odd).
            v = arr.reshape(D, TP // 2, 2, 2, 128)   # [d, p2, e, ib, m]
            sw = v[:, :, :, :, ::-1].transpose(0, 1, 3, 4, 2)
            xpts.append(np.ascontiguousarray(sw).reshape(D, TP // 2, 2, 256))
        else:
            xpts.append(np.ascontiguousarray(arr))
    yts = [np.ascontiguousarray(
        Y8[b * NJ:(b + 1) * NJ].transpose(2, 1, 0)) for b in range(GB)]
    in_maps = [{"xpt": xpts[k // GB], "yt": yts[k % GB]}
               for k in range(N_CORES)]

    trace = bool(os.environ.get("BASS_TRACE"))
    LAST_RUN = run_bass_kernel_spmd(nc, in_maps, list(range(N_CORES)),
                                    trace=trace)
    C3 = np.empty((NX, NY), dtype=np.float32)
    for k in range(N_CORES):
        a, b = k // GB, k % GB
        C3[a * NLI:(a + 1) * NLI, b * NJ:(b + 1) * NJ] = \
            LAST_RUN.results[k]["c3"]

    # Host epilogue: rank-1 corrections (0.15% of FLOPs).
    row_c = pi.sum(-1)                                 # [C, T]
    col_c = pi.sum(1)                                  # [C, TP]
    SX = np.einsum("itd,itd->it", X, X)                # [NX, T]
    SY = np.einsum("jpd,jpd->jp", Y, Y)                # [NY, TP]
    C1 = np.einsum("it,it->i", SX, row_c[classe])      # [NX]
    C2 = col_c @ SY.T                                  # [C, NY]
    return (C1[:, None] + C2[classe] - 2.0 * C3).astype(np.float32)


# revision 8
# speedup vs baseline: 2.4700x; 1.2791x over previous
"""Fused OT-DTW l2 cost-matrix kernel for Trainium2 (8 NeuronCores, SPMD).

mat_cost[i,j] = sum_{t,p,d} pi[cl(i)][t,p] * (X[i,t,d] - Y[j,p,d])^2
             = C1[i] + C2[cl(i), j] - 2 * C3[i,j]

with C3[i,j] = sum_{p,d} XP[i,p,d] * Y[j,p,d],  XP[i] = X[i].T @ pi[cl(i)].

Key structural fact: pi[c] is a monotone DTW *path* matrix, so each column p
covers a contiguous t-interval [t0(p), t1(p)]. Hence
    XP[i,p,:] = SX[i, t1(p)+1, :] - SX[i, t0(p), :],   SX = cumsum_t(X[i])
— the 69-GFLOP "stage A" collapses to a prefix-sum + gather-diff, done
exactly in f32 on the host (~100 MFLOP). The device runs only the heavy
C3 contraction (137 GFLOP) in fp8 DoubleRowSwInterleave.

Sharding: 2 i-groups x 4 p-quarters — core k=(a,b) computes the PARTIAL
C3 over p in [128b, 128b+128) for rows [512a, 512a+512), all 1024 j; the
host sums the 4 partials per i-group (f32, cheap). Per-core HBM traffic:
xpt 8.4MB + yt 16.8MB + out 2MB (vs 50.3MB for (i,j) 4x2 and 75.5MB for
1D) against a ~430 GB/s per-core DMA ceiling. Full j-width also lets one
weight load (p-pair, i-block) feed both N=512 matmuls (j-halves): the
second sets InstMatmult.ldweights=False to skip the redundant LDWEIGHTS
(~145ns each, the dominant PE overhead at N=512 with DoubleRow).

Host pre-permutes so every DMA is contiguous:
  xpt [d, pp, ib, 256]  fp8 SWI weight layout (pairs interleaved, columns
                        reversed: A127,B127,...,A0,B0), resident in SBUF
  yt  [d, p, j]         fp8, streamed in 1MB tiles through an 8-buf ring
Per core: scratch-matmul warmup for the PE clock-gate, then 64 p-pairs x
4 i-blocks x 2 j-halves of DoubleRowSwInterleave matmuls into 8 PSUM
banks. The rank-1 corrections C1/C2 are applied on the host in fp32.
"""

import os
import sys
import types

import numpy as np
import ml_dtypes

NX, NY, T, TP, D, C = 1024, 1024, 512, 512, 128, 8
N_CORES = 8
GA, GB = 2, 4               # core grid: 2 i-groups x 4 p-quarters
NLI = NX // GA              # 512 rows per core
NIB = NLI // 128            # 4 i-blocks
QP = TP // GB               # 128 p-values per core
QPAIR = QP // 2             # 64 p-pairs per core
PG = 8                      # p-values per Y DMA tile (1MB)
NT = QP // PG               # 16 y tiles
F8 = ml_dtypes.float8_e4m3fn

SHARE_W = True   # skip LDWEIGHTS on the second j-half matmul of each pair


def _ensure_axon_hooks():
    """concourse.bass_utils imports antenv.axon_hooks when tracing under
    axon; some images lack that submodule. Provide it, and register the
    NTFF profile hook if the boot path didn't."""
    try:
        import antenv
    except ImportError:
        return
    try:
        from antenv import axon_hooks  # noqa: F401
    except ImportError:
        mod = types.ModuleType("antenv.axon_hooks")
        mod._hook = None

        def _set(h):
            mod._hook = h

        def _get():
            return mod._hook

        mod.set_axon_ntff_profile_hook = _set
        mod.get_axon_ntff_profile_hook = _get
        sys.modules["antenv.axon_hooks"] = mod
        antenv.axon_hooks = mod
    from antenv.axon_hooks import (
        get_axon_ntff_profile_hook,
        set_axon_ntff_profile_hook,
    )

    if get_axon_ntff_profile_hook() is None:
        try:
            from trn_agent_boot.trn_boot import _ntff_profile_via_ctypes

            hook = _ntff_profile_via_ctypes("/opt/axon/libaxon_pjrt.so")
            if hook is not None:
                set_axon_ntff_profile_hook(hook)
        except Exception:
            pass


_ensure_axon_hooks()

import concourse.bass as bass  # noqa: E402, F401
import concourse.tile as tile  # noqa: E402
from concourse import bacc, mybir  # noqa: E402
from concourse.bass_utils import run_bass_kernel_spmd  # noqa: E402

_PROGRAM_CACHE = {}
LAST_RUN = None  # BassKernelResults of the most recent kernel() call


def _build_program():
    if "nc" in _PROGRAM_CACHE:
        return _PROGRAM_CACHE["nc"]
    f8 = mybir.dt.float8e4
    f32 = mybir.dt.float32
    PM = mybir.MatmulPerfMode.DoubleRowSwInterleave
    nc = bacc.Bacc("TRN2", target_bir_lowering=False, debug=False,
                   num_devices=N_CORES)
    xpt = nc.dram_tensor("xpt", [D, QPAIR, NIB, 256], f8,
                         kind="ExternalInput").ap()
    yt = nc.dram_tensor("yt", [D, QP, NY], f8, kind="ExternalInput").ap()
    c3 = nc.dram_tensor("c3", [NLI, NY], f32, kind="ExternalOutput").ap()

    with tile.TileContext(nc) as tc:
        with (
            tc.tile_pool(name="xptsb", bufs=1) as xpt_pool,
            tc.tile_pool(name="yin", bufs=8) as y_pool,
            tc.tile_pool(name="outsb", bufs=1) as out_pool,
        ):
            xpt_sb = xpt_pool.tile([D, QPAIR, NIB, 256], f8)

            # PE warmup: scratch matmuls at t=0, overlapping the first input
            # DMAs, so the HAM clock-gate reaches full speed before the real
            # matmuls start (values never read).
            with (
                tc.tile_pool(name="warm", bufs=1) as warm_pool,
                tc.tile_pool(name="warmps", bufs=1, space="PSUM") as warmps_pool,
            ):
                wsrc = warm_pool.tile([128, 512], f8)
                wacc = warmps_pool.tile([128, 512], f32)
                nc.gpsimd.memset(wsrc[:], 0.0)
                for w in range(14):
                    nc.tensor.matmul(wacc[:], wsrc[:, 0:128], wsrc[:],
                                     start=True, stop=True)

            # xpt chunk c holds the weights for y tile c's pairs; keep one
            # tile of lookahead. Alternate every DMA between the two HWDGE
            # rings (sync=SP, scalar=ACT).
            rings = [nc.sync, nc.scalar]

            def xchunk(c, ring):
                ring.dma_start(xpt_sb[:, c * (PG // 2):(c + 1) * (PG // 2)],
                               xpt[:, c * (PG // 2):(c + 1) * (PG // 2)])

            xchunk(0, nc.sync)
            xchunk(1, nc.scalar)

            with tc.tile_pool(name="ps", bufs=1, space="PSUM") as ps_pool:
                accs = [ps_pool.tile([128, 512], f32, name=f"acc{i}")
                        for i in range(2 * NIB)]  # [ib*2 + jhalf]
                for g in range(NT):
                    ytile = y_pool.tile([D, PG, NY], f8)
                    rings[g % 2].dma_start(
                        ytile[:], yt[:, g * PG:(g + 1) * PG, :])
                    if g + 2 < NT:
                        xchunk(g + 2, rings[(g + 1) % 2])
                    for s in range(PG // 2):
                        pp = g * (PG // 2) + s
                        st, sp = (pp == 0), (pp == QPAIR - 1)
                        rhs0 = ytile[:, 2 * s:2 * s + 2, 0:512]
                        rhs1 = ytile[:, 2 * s:2 * s + 2, 512:1024]
                        for ib in range(NIB):
                            w = xpt_sb[:, pp, ib, :]
                            nc.tensor.matmul(accs[2 * ib][:], w, rhs0,
                                             start=st, stop=sp, perf_mode=PM)
                            i1 = nc.tensor.matmul(accs[2 * ib + 1][:], w, rhs1,
                                                  start=st, stop=sp,
                                                  perf_mode=PM)
                            if SHARE_W:
                                i1.ldweights = False

            out_sb = out_pool.tile([128, NIB, NY], f32)
            for ib in range(NIB):
                nc.vector.tensor_copy(out_sb[:, ib, 0:512], accs[2 * ib][:])
                nc.scalar.copy(out_sb[:, ib, 512:1024], accs[2 * ib + 1][:])
            for ib in range(NIB):
                rings[ib % 2].dma_start(c3[128 * ib:128 * (ib + 1), :],
                                        out_sb[:, ib, :])

    nc.compile()
    _PROGRAM_CACHE["nc"] = nc
    return nc


def _host_xp(X, pi, classe):
    """XP[i,p,:] = sum_t pi[cl(i)][t,p] * X[i,t,:], exact f32.

    Fast path uses the DTW-path structure (each pi column covers a
    contiguous t-interval): prefix sums + gather-diff. Falls back to
    per-class BLAS if any column is non-contiguous or empty."""
    nz = pi > 0.5                                    # [C, T, TP]
    cnt = nz.sum(axis=1)                             # [C, TP]
    tidx = np.arange(T, dtype=np.int64)[None, :, None]
    t1 = np.where(nz, tidx, -1).max(axis=1)          # [C, TP]
    t0 = np.where(nz, tidx, T).min(axis=1)           # [C, TP]
    if (cnt > 0).all() and (cnt == t1 - t0 + 1).all():
        SX = np.zeros((NX, T + 1, D), dtype=np.float32)
        np.cumsum(X, axis=1, out=SX[:, 1:, :])
        ar = np.arange(NX)[:, None]
        return SX[ar, t1[classe] + 1, :] - SX[ar, t0[classe], :]
    XP = np.empty((NX, TP, D), dtype=np.float32)
    for c in range(C):
        m = classe == c
        if m.any():
            # [n,D,T] @ [T,TP] -> [n,D,TP] -> [n,TP,D]
            XP[m] = np.matmul(X[m].transpose(0, 2, 1), pi[c]).transpose(0, 2, 1)
    return XP


def _sw_weights(xp8_rows, pq):
    """SWI weight layout for one (i-group, p-quarter): [D, QPAIR, NIB, 256].

    Per (p-pair, i-block) the 256-byte weight stream is
    [A(127), B(127), ..., A(0), B(0)] — pairs interleaved, columns
    reversed (A = even p of the pair, B = odd)."""
    arr = xp8_rows[:, pq * QP:(pq + 1) * QP, :].transpose(2, 1, 0)
    v = arr.reshape(D, QPAIR, 2, NIB, 128)           # [d, pp, e, ib, m]
    sw = v[:, :, :, :, ::-1].transpose(0, 1, 3, 4, 2)
    return np.ascontiguousarray(sw).reshape(D, QPAIR, NIB, 256)


def kernel(X, Y, pi, classe):
    global LAST_RUN
    assert X.shape == (NX, T, D) and Y.shape == (NY, TP, D)
    assert pi.shape == (C, T, TP) and classe.shape == (NX,)
    X = np.asarray(X, dtype=np.float32)
    Y = np.asarray(Y, dtype=np.float32)
    pi = np.asarray(pi, dtype=np.float32)
    classe = np.asarray(classe)

    nc = _build_program()

    XP8 = _host_xp(X, pi, classe).astype(F8)         # [NX, TP, D]
    Y8 = Y.astype(F8)                                # [NY, TP, D]
    yts = [np.ascontiguousarray(
        Y8[:, b * QP:(b + 1) * QP, :].transpose(2, 1, 0)) for b in range(GB)]
    in_maps = []
    for k in range(N_CORES):
        a, b = k // GB, k % GB
        in_maps.append(
            {"xpt": _sw_weights(XP8[a * NLI:(a + 1) * NLI], b), "yt": yts[b]})

    trace = bool(os.environ.get("BASS_TRACE"))
    LAST_RUN = run_bass_kernel_spmd(nc, in_maps, list(range(N_CORES)),
                                    trace=trace)
    C3 = np.empty((NX, NY), dtype=np.float32)
    for a in range(GA):
        acc = LAST_RUN.results[a * GB]["c3"].astype(np.float32)
        for b in range(1, GB):
            acc += LAST_RUN.results[a * GB + b]["c3"]
        C3[a * NLI:(a + 1) * NLI] = acc

    # Host epilogue: rank-1 corrections (0.15% of FLOPs).
    row_c = pi.sum(-1)                                 # [C, T]
    col_c = pi.sum(1)                                  # [C, TP]
    SX = np.einsum("itd,itd->it", X, X)                # [NX, T]
    SY = np.einsum("jpd,jpd->jp", Y, Y)                # [NY, TP]
    C1 = np.einsum("it,it->i", SX, row_c[classe])      # [NX]
    C2 = col_c @ SY.T                                  # [C, NY]
    return (C1[:, None] + C2[classe] - 2.0 * C3).astype(np.float32)


# revision 18
# speedup vs baseline: 2.5186x; 1.0197x over previous
"""Fused OT-DTW l2 cost-matrix kernel for Trainium2 (8 NeuronCores, SPMD).

mat_cost[i,j] = sum_{t,p,d} pi[cl(i)][t,p] * (X[i,t,d] - Y[j,p,d])^2
             = C1[i] + C2[cl(i), j] - 2 * C3[i,j]

with C3[i,j] = sum_{p,d} XP[i,p,d] * Y[j,p,d],  XP[i] = X[i].T @ pi[cl(i)].

Key structural fact: pi[c] is a monotone DTW *path* matrix, so each column p
covers a contiguous t-interval [t0(p), t1(p)]. Hence
    XP[i,p,:] = SX[i, t1(p)+1, :] - SX[i, t0(p), :],   SX = cumsum_t(X[i])
— the 69-GFLOP "stage A" collapses to a prefix-sum + gather-diff, done
exactly in f32 on the host (~100 MFLOP). The device runs only the heavy
C3 contraction (137 GFLOP) in fp8 DoubleRowSwInterleave.

Sharding: 2 i-groups x 4 p-quarters — core k=(a,b) computes the PARTIAL
C3 over p in [128b, 128b+128) for rows [512a, 512a+512), all 1024 j; the
host sums the 4 partials per i-group (f32, cheap). Per-core HBM traffic:
xpt 8.4MB + yt 16.8MB + out 2MB (vs 50.3MB for (i,j) 4x2 and 75.5MB for
1D) against a ~430 GB/s per-core DMA ceiling. Full j-width also lets one
weight load (p-pair, i-block) feed both N=512 matmuls (j-halves): the
second sets InstMatmult.ldweights=False to skip the redundant LDWEIGHTS
(~145ns each, the dominant PE overhead at N=512 with DoubleRow).

Host pre-permutes so every DMA is contiguous:
  xpt [d, pp, ib, 256]  fp8 SWI weight layout (pairs interleaved, columns
                        reversed: A127,B127,...,A0,B0), resident in SBUF
  yt  [d, p, j]         fp8, streamed in 1MB tiles through an 8-buf ring
Per core: scratch-matmul warmup for the PE clock-gate, then 64 p-pairs x
4 i-blocks x 2 j-halves of DoubleRowSwInterleave matmuls into 8 PSUM
banks. The rank-1 corrections C1/C2 are applied on the host in fp32.
"""

import json
import os
import sys
import types

import numpy as np
import ml_dtypes

NX, NY, T, TP, D, C = 1024, 1024, 512, 512, 128, 8
N_CORES = 8
GA, GB = 2, 4               # core grid: 2 i-groups x 4 p-quarters
NLI = NX // GA              # 512 rows per core
NIB = NLI // 128            # 4 i-blocks
QP = TP // GB               # 128 p-values per core
QPAIR = QP // 2             # 64 p-pairs per core
PG = 8                      # p-values per Y DMA tile (1MB)
NT = QP // PG               # 16 y tiles
F8 = ml_dtypes.float8_e4m3fn

SHARE_W = False  # drop duplicate LDWEIGHTS via module JSON surgery (hangs HW)
ROUNDTRIP = True  # inert module JSON round-trip (validates the surgery path)


def _ensure_axon_hooks():
    """concourse.bass_utils imports antenv.axon_hooks when tracing under
    axon; some images lack that submodule. Provide it, and register the
    NTFF profile hook if the boot path didn't."""
    try:
        import antenv
    except ImportError:
        return
    try:
        from antenv import axon_hooks  # noqa: F401
    except ImportError:
        mod = types.ModuleType("antenv.axon_hooks")
        mod._hook = None

        def _set(h):
            mod._hook = h

        def _get():
            return mod._hook

        mod.set_axon_ntff_profile_hook = _set
        mod.get_axon_ntff_profile_hook = _get
        sys.modules["antenv.axon_hooks"] = mod
        antenv.axon_hooks = mod
    from antenv.axon_hooks import (
        get_axon_ntff_profile_hook,
        set_axon_ntff_profile_hook,
    )

    if get_axon_ntff_profile_hook() is None:
        try:
            from trn_agent_boot.trn_boot import _ntff_profile_via_ctypes

            hook = _ntff_profile_via_ctypes("/opt/axon/libaxon_pjrt.so")
            if hook is not None:
                set_axon_ntff_profile_hook(hook)
        except Exception:
            pass


_ensure_axon_hooks()

import concourse.bass as bass  # noqa: E402, F401
import concourse.tile as tile  # noqa: E402
from concourse import bacc, mybir  # noqa: E402
from concourse.bass_utils import run_bass_kernel_spmd  # noqa: E402

_PROGRAM_CACHE = {}
LAST_RUN = None  # BassKernelResults of the most recent kernel() call


def _build_program():
    if "nc" in _PROGRAM_CACHE:
        return _PROGRAM_CACHE["nc"]
    f8 = mybir.dt.float8e4
    f32 = mybir.dt.float32
    PM = mybir.MatmulPerfMode.DoubleRowSwInterleave
    nc = bacc.Bacc("TRN2", target_bir_lowering=False, debug=False,
                   num_devices=N_CORES)
    bf16 = mybir.dt.bfloat16
    xpt = nc.dram_tensor("xpt", [D, QPAIR, NIB, 256], f8,
                         kind="ExternalInput").ap()
    yt = nc.dram_tensor("yt", [D, QP, NY], f8, kind="ExternalInput").ap()
    c3 = nc.dram_tensor("c3", [NLI, NY], bf16, kind="ExternalOutput").ap()

    with tile.TileContext(nc) as tc:
        with (
            tc.tile_pool(name="xptsb", bufs=1) as xpt_pool,
            tc.tile_pool(name="yin", bufs=10) as y_pool,
            tc.tile_pool(name="outsb", bufs=1) as out_pool,
        ):
            xpt_sb = xpt_pool.tile([D, QPAIR, NIB, 256], f8)

            # PE warmup: scratch matmuls at t=0, overlapping the first input
            # DMAs, so the HAM clock-gate reaches full speed before the real
            # matmuls start (values never read).
            with (
                tc.tile_pool(name="warm", bufs=1) as warm_pool,
                tc.tile_pool(name="warmps", bufs=1, space="PSUM") as warmps_pool,
            ):
                wsrc = warm_pool.tile([128, 512], f8)
                wacc = warmps_pool.tile([128, 512], f32)
                nc.gpsimd.memset(wsrc[:], 0.0)
                for w in range(14):
                    nc.tensor.matmul(wacc[:], wsrc[:, 0:128], wsrc[:],
                                     start=True, stop=True)

            # Dedicated rings: yt stream on sync (SP HWDGE), xpt chunks +
            # output on scalar (ACT HWDGE) — so neither stream queues
            # behind the other. xpt chunk c holds the weights for y tile
            # c's pairs; keep one tile of lookahead.
            def xchunk(c):
                nc.scalar.dma_start(
                    xpt_sb[:, c * (PG // 2):(c + 1) * (PG // 2)],
                    xpt[:, c * (PG // 2):(c + 1) * (PG // 2)])

            xchunk(0)
            xchunk(1)

            with tc.tile_pool(name="ps", bufs=1, space="PSUM") as ps_pool:
                accs = [ps_pool.tile([128, 512], f32, name=f"acc{i}")
                        for i in range(2 * NIB)]  # [ib*2 + jhalf]
                for g in range(NT):
                    ytile = y_pool.tile([D, PG, NY], f8)
                    nc.sync.dma_start(
                        ytile[:], yt[:, g * PG:(g + 1) * PG, :])
                    if g + 2 < NT:
                        xchunk(g + 2)
                    for s in range(PG // 2):
                        pp = g * (PG // 2) + s
                        st, sp = (pp == 0), (pp == QPAIR - 1)
                        rhs0 = ytile[:, 2 * s:2 * s + 2, 0:512]
                        rhs1 = ytile[:, 2 * s:2 * s + 2, 512:1024]
                        for ib in range(NIB):
                            w = xpt_sb[:, pp, ib, :]
                            nc.tensor.matmul(accs[2 * ib][:], w, rhs0,
                                             start=st, stop=sp, perf_mode=PM)
                            nc.tensor.matmul(accs[2 * ib + 1][:], w, rhs1,
                                             start=st, stop=sp, perf_mode=PM)

            out_sb = out_pool.tile([128, NIB, NY], bf16)
            for ib in range(NIB):
                nc.vector.tensor_copy(out_sb[:, ib, 0:512], accs[2 * ib][:])
                nc.scalar.copy(out_sb[:, ib, 512:1024], accs[2 * ib + 1][:])
            for ib in range(NIB):
                eng = nc.sync if ib % 2 == 0 else nc.scalar
                eng.dma_start(c3[128 * ib:128 * (ib + 1), :],
                              out_sb[:, ib, :])

    nc.compile()
    if SHARE_W:
        n = _dedupe_ldweights(nc)
        assert n >= QPAIR * NIB, n  # 256 pair-dups (+ warmup dups)
    elif ROUNDTRIP:
        nc.m = mybir.module_from_json_string(mybir.module_to_json_string(nc.m))
    _PROGRAM_CACHE["nc"] = nc
    return nc


def _dedupe_ldweights(nc):
    """Drop InstLdweights that reload the exact weights already in the PE
    array (identical AP as the previous Ldweights, no semaphore traffic).
    The following Matmult already carries ldweights=False and streams with
    the loaded weights — the documented standalone-LDW pairing. Saves
    ~145ns per deleted load; tile_legalize otherwise re-emits one per
    matmul unconditionally."""
    d = json.loads(mybir.module_to_json_string(nc.m))
    removed = 0
    for fn in d["functions"]:
        for blk in fn["blocks"]:
            out = []
            last_w = None
            for inst in blk["instructions"]:
                if inst.get("opcode") == "Ldweights":
                    key = json.dumps(
                        [inst.get("ins"), inst.get("perf_mode"),
                         inst.get("tile_position"), inst.get("tile_size"),
                         inst.get("is_transpose")], sort_keys=True)
                    sync = inst.get("sync_info") or {}
                    if (key == last_w and not sync.get("on_wait")
                            and not sync.get("on_update")):
                        removed += 1
                        continue
                    last_w = key
                out.append(inst)
            blk["instructions"] = out
    nc.m = mybir.module_from_json_string(json.dumps(d))
    return removed


def _host_xp(X, pi, classe):
    """XP[i,p,:] = sum_t pi[cl(i)][t,p] * X[i,t,:], exact f32.

    Fast path uses the DTW-path structure (each pi column covers a
    contiguous t-interval): prefix sums + gather-diff. Falls back to
    per-class BLAS if any column is non-contiguous or empty."""
    nz = pi > 0.5                                    # [C, T, TP]
    cnt = nz.sum(axis=1)                             # [C, TP]
    tidx = np.arange(T, dtype=np.int64)[None, :, None]
    t1 = np.where(nz, tidx, -1).max(axis=1)          # [C, TP]
    t0 = np.where(nz, tidx, T).min(axis=1)           # [C, TP]
    if (cnt > 0).all() and (cnt == t1 - t0 + 1).all():
        SX = np.zeros((NX, T + 1, D), dtype=np.float32)
        np.cumsum(X, axis=1, out=SX[:, 1:, :])
        ar = np.arange(NX)[:, None]
        return SX[ar, t1[classe] + 1, :] - SX[ar, t0[classe], :]
    XP = np.empty((NX, TP, D), dtype=np.float32)
    for c in range(C):
        m = classe == c
        if m.any():
            # [n,D,T] @ [T,TP] -> [n,D,TP] -> [n,TP,D]
            XP[m] = np.matmul(X[m].transpose(0, 2, 1), pi[c]).transpose(0, 2, 1)
    return XP


def _sw_weights(xp8_rows, pq):
    """SWI weight layout for one (i-group, p-quarter): [D, QPAIR, NIB, 256].

    Per (p-pair, i-block) the 256-byte weight stream is
    [A(127), B(127), ..., A(0), B(0)] — pairs interleaved, columns
    reversed (A = even p of the pair, B = odd)."""
    arr = xp8_rows[:, pq * QP:(pq + 1) * QP, :].transpose(2, 1, 0)
    v = arr.reshape(D, QPAIR, 2, NIB, 128)           # [d, pp, e, ib, m]
    sw = v[:, :, :, :, ::-1].transpose(0, 1, 3, 4, 2)
    return np.ascontiguousarray(sw).reshape(D, QPAIR, NIB, 256)


def kernel(X, Y, pi, classe):
    global LAST_RUN
    assert X.shape == (NX, T, D) and Y.shape == (NY, TP, D)
    assert pi.shape == (C, T, TP) and classe.shape == (NX,)
    X = np.asarray(X, dtype=np.float32)
    Y = np.asarray(Y, dtype=np.float32)
    pi = np.asarray(pi, dtype=np.float32)
    classe = np.asarray(classe)

    nc = _build_program()

    XP8 = _host_xp(X, pi, classe).astype(F8)         # [NX, TP, D]
    Y8 = Y.astype(F8)                                # [NY, TP, D]
    yts = [np.ascontiguousarray(
        Y8[:, b * QP:(b + 1) * QP, :].transpose(2, 1, 0)) for b in range(GB)]
    in_maps = []
    for k in range(N_CORES):
        a, b = k // GB, k % GB
        in_maps.append(
            {"xpt": _sw_weights(XP8[a * NLI:(a + 1) * NLI], b), "yt": yts[b]})

    trace = bool(os.environ.get("BASS_TRACE"))
    LAST_RUN = run_bass_kernel_spmd(nc, in_maps, list(range(N_CORES)),
                                    trace=trace)
    C3 = np.empty((NX, NY), dtype=np.float32)
    for a in range(GA):
        acc = LAST_RUN.results[a * GB]["c3"].astype(np.float32)
        for b in range(1, GB):
            acc += LAST_RUN.results[a * GB + b]["c3"]
        C3[a * NLI:(a + 1) * NLI] = acc

    # Host epilogue: rank-1 corrections (0.15% of FLOPs).
    row_c = pi.sum(-1)                                 # [C, T]
    col_c = pi.sum(1)                                  # [C, TP]
    SX = np.einsum("itd,itd->it", X, X)                # [NX, T]
    SY = np.einsum("jpd,jpd->jp", Y, Y)                # [NY, TP]
    C1 = np.einsum("it,it->i", SX, row_c[classe])      # [NX]
    C2 = col_c @ SY.T                                  # [C, NY]
    return (C1[:, None] + C2[classe] - 2.0 * C3).astype(np.float32)


# revision 23
# speedup vs baseline: 2.5913x; 1.0289x over previous
"""Fused OT-DTW l2 cost-matrix kernel for Trainium2 (8 NeuronCores, SPMD).

mat_cost[i,j] = sum_{t,p,d} pi[cl(i)][t,p] * (X[i,t,d] - Y[j,p,d])^2
             = C1[i] + C2[cl(i), j] - 2 * C3[i,j]

with C3[i,j] = sum_{p,d} XP[i,p,d] * Y[j,p,d],  XP[i] = X[i].T @ pi[cl(i)].

Key structural fact: pi[c] is a monotone DTW *path* matrix, so each column p
covers a contiguous t-interval [t0(p), t1(p)]. Hence
    XP[i,p,:] = SX[i, t1(p)+1, :] - SX[i, t0(p), :],   SX = cumsum_t(X[i])
— the 69-GFLOP "stage A" collapses to a prefix-sum + gather-diff, done
exactly in f32 on the host (~100 MFLOP). The device runs only the heavy
C3 contraction (137 GFLOP) in fp8 DoubleRowSwInterleave.

Sharding: 2 i-groups x 4 p-quarters — core k=(a,b) computes the PARTIAL
C3 over p in [128b, 128b+128) for rows [512a, 512a+512), all 1024 j; the
host sums the 4 partials per i-group (f32, cheap). Per-core HBM traffic:
xpt 8.4MB + yt 16.8MB + out 2MB (vs 50.3MB for (i,j) 4x2 and 75.5MB for
1D) against a ~430 GB/s per-core DMA ceiling. Full j-width also lets one
weight load (p-pair, i-block) feed both N=512 matmuls (j-halves): the
second sets InstMatmult.ldweights=False to skip the redundant LDWEIGHTS
(~145ns each, the dominant PE overhead at N=512 with DoubleRow).

Host pre-permutes so every DMA is contiguous:
  xpt [d, pp, ib, 256]  fp8 SWI weight layout (pairs interleaved, columns
                        reversed: A127,B127,...,A0,B0), resident in SBUF
  yt  [d, p, j]         fp8, streamed in 1MB tiles through an 8-buf ring
Per core: scratch-matmul warmup for the PE clock-gate, then 64 p-pairs x
4 i-blocks x 2 j-halves of DoubleRowSwInterleave matmuls into 8 PSUM
banks. The rank-1 corrections C1/C2 are applied on the host in fp32.
"""

import json
import os
import sys
import types

import numpy as np
import ml_dtypes

NX, NY, T, TP, D, C = 1024, 1024, 512, 512, 128, 8
N_CORES = 8
GA, GB = 2, 4               # core grid: 2 i-groups x 4 p-quarters
NLI = NX // GA              # 512 rows per core
NIB = NLI // 128            # 4 i-blocks
QP = TP // GB               # 128 p-values per core
QPAIR = QP // 2             # 64 p-pairs per core
PG = 8                      # p-values per Y DMA tile (1MB)
NT = QP // PG               # 16 y tiles
F8 = ml_dtypes.float8_e4m3fn

SHARE_W = False  # drop duplicate LDWEIGHTS via module JSON surgery (hangs HW)
ROUNDTRIP = True  # inert module JSON round-trip (validates the surgery path)


def _ensure_axon_hooks():
    """concourse.bass_utils imports antenv.axon_hooks when tracing under
    axon; some images lack that submodule. Provide it, and register the
    NTFF profile hook if the boot path didn't."""
    try:
        import antenv
    except ImportError:
        return
    try:
        from antenv import axon_hooks  # noqa: F401
    except ImportError:
        mod = types.ModuleType("antenv.axon_hooks")
        mod._hook = None

        def _set(h):
            mod._hook = h

        def _get():
            return mod._hook

        mod.set_axon_ntff_profile_hook = _set
        mod.get_axon_ntff_profile_hook = _get
        sys.modules["antenv.axon_hooks"] = mod
        antenv.axon_hooks = mod
    from antenv.axon_hooks import (
        get_axon_ntff_profile_hook,
        set_axon_ntff_profile_hook,
    )

    if get_axon_ntff_profile_hook() is None:
        try:
            from trn_agent_boot.trn_boot import _ntff_profile_via_ctypes

            hook = _ntff_profile_via_ctypes("/opt/axon/libaxon_pjrt.so")
            if hook is not None:
                set_axon_ntff_profile_hook(hook)
        except Exception:
            pass


_ensure_axon_hooks()

import concourse.bass as bass  # noqa: E402, F401
import concourse.tile as tile  # noqa: E402
from concourse import bacc, mybir  # noqa: E402
from concourse.bass_utils import run_bass_kernel_spmd  # noqa: E402

_PROGRAM_CACHE = {}
LAST_RUN = None  # BassKernelResults of the most recent kernel() call


def _build_program():
    if "nc" in _PROGRAM_CACHE:
        return _PROGRAM_CACHE["nc"]
    f8 = mybir.dt.float8e4
    f32 = mybir.dt.float32
    PM = mybir.MatmulPerfMode.DoubleRowSwInterleave
    nc = bacc.Bacc("TRN2", target_bir_lowering=False, debug=False,
                   num_devices=N_CORES)
    bf16 = mybir.dt.bfloat16
    xpt = nc.dram_tensor("xpt", [D, QPAIR, NIB, 256], f8,
                         kind="ExternalInput").ap()
    yt = nc.dram_tensor("yt", [D, QP, NY], f8, kind="ExternalInput").ap()
    c3 = nc.dram_tensor("c3", [NLI, NY], bf16, kind="ExternalOutput").ap()

    with tile.TileContext(nc) as tc:
        with (
            tc.tile_pool(name="xptsb", bufs=1) as xpt_pool,
            tc.tile_pool(name="yin", bufs=4) as y_pool,
            tc.tile_pool(name="outsb", bufs=1) as out_pool,
        ):
            xpt_sb = xpt_pool.tile([D, QPAIR, NIB, 256], f8)

            # PE warmup: scratch matmuls at t=0, overlapping the first input
            # DMAs, so the HAM clock-gate reaches full speed before the real
            # matmuls start (values never read).
            with (
                tc.tile_pool(name="warm", bufs=1) as warm_pool,
                tc.tile_pool(name="warmps", bufs=1, space="PSUM") as warmps_pool,
            ):
                wsrc = warm_pool.tile([128, 512], f8)
                wacc = warmps_pool.tile([128, 512], f32)
                nc.gpsimd.memset(wsrc[:], 0.0)
                for w in range(14):
                    nc.tensor.matmul(wacc[:], wsrc[:, 0:128], wsrc[:],
                                     start=True, stop=True)

            # Dedicated rings: yt stream on sync (SP HWDGE), xpt chunks +
            # output on scalar (ACT HWDGE) — so neither stream queues
            # behind the other. xpt chunk c holds the weights for y tile
            # c's pairs; keep one tile of lookahead.
            def xchunk(c):
                nc.scalar.dma_start(
                    xpt_sb[:, c * (PG // 2):(c + 1) * (PG // 2)],
                    xpt[:, c * (PG // 2):(c + 1) * (PG // 2)])

            for c in range(4):
                xchunk(c)

            with tc.tile_pool(name="ps", bufs=1, space="PSUM") as ps_pool:
                accs = [ps_pool.tile([128, 512], f32, name=f"acc{i}")
                        for i in range(2 * NIB)]  # [ib*2 + jhalf]
                for g in range(NT):
                    ytile = y_pool.tile([D, PG, NY], f8)
                    if g == 0:
                        # Split tile 0 so the first matmuls wait on only
                        # half the bytes (slice-level deps).
                        h = PG // 2
                        nc.sync.dma_start(ytile[:, 0:h], yt[:, 0:h, :])
                        nc.sync.dma_start(ytile[:, h:PG], yt[:, h:PG, :])
                    else:
                        nc.sync.dma_start(
                            ytile[:], yt[:, g * PG:(g + 1) * PG, :])
                    if g + 4 < NT:
                        xchunk(g + 4)
                    for s in range(PG // 2):
                        pp = g * (PG // 2) + s
                        st, sp = (pp == 0), (pp == QPAIR - 1)
                        rhs0 = ytile[:, 2 * s:2 * s + 2, 0:512]
                        rhs1 = ytile[:, 2 * s:2 * s + 2, 512:1024]
                        for ib in range(NIB):
                            w = xpt_sb[:, pp, ib, :]
                            nc.tensor.matmul(accs[2 * ib][:], w, rhs0,
                                             start=st, stop=sp, perf_mode=PM)
                            nc.tensor.matmul(accs[2 * ib + 1][:], w, rhs1,
                                             start=st, stop=sp, perf_mode=PM)

            out_sb = out_pool.tile([128, NIB, NY], bf16)
            for ib in range(NIB):
                nc.vector.tensor_copy(out_sb[:, ib, 0:512], accs[2 * ib][:])
                nc.scalar.copy(out_sb[:, ib, 512:1024], accs[2 * ib + 1][:])
            for ib in range(NIB):
                eng = nc.sync if ib % 2 == 0 else nc.scalar
                eng.dma_start(c3[128 * ib:128 * (ib + 1), :],
                              out_sb[:, ib, :])

    nc.compile()
    if SHARE_W:
        n = _dedupe_ldweights(nc)
        assert n >= QPAIR * NIB, n  # 256 pair-dups (+ warmup dups)
    elif ROUNDTRIP:
        nc.m = mybir.module_from_json_string(mybir.module_to_json_string(nc.m))
    _PROGRAM_CACHE["nc"] = nc
    return nc


def _dedupe_ldweights(nc):
    """Drop InstLdweights that reload the exact weights already in the PE
    array (identical AP as the previous Ldweights, no semaphore traffic).
    The following Matmult already carries ldweights=False and streams with
    the loaded weights — the documented standalone-LDW pairing. Saves
    ~145ns per deleted load; tile_legalize otherwise re-emits one per
    matmul unconditionally."""
    d = json.loads(mybir.module_to_json_string(nc.m))
    removed = 0
    for fn in d["functions"]:
        for blk in fn["blocks"]:
            out = []
            last_w = None
            for inst in blk["instructions"]:
                if inst.get("opcode") == "Ldweights":
                    key = json.dumps(
                        [inst.get("ins"), inst.get("perf_mode"),
                         inst.get("tile_position"), inst.get("tile_size"),
                         inst.get("is_transpose")], sort_keys=True)
                    sync = inst.get("sync_info") or {}
                    if (key == last_w and not sync.get("on_wait")
                            and not sync.get("on_update")):
                        removed += 1
                        continue
                    last_w = key
                out.append(inst)
            blk["instructions"] = out
    nc.m = mybir.module_from_json_string(json.dumps(d))
    return removed


def _host_xp(X, pi, classe):
    """XP[i,p,:] = sum_t pi[cl(i)][t,p] * X[i,t,:], exact f32.

    Fast path uses the DTW-path structure (each pi column covers a
    contiguous t-interval): prefix sums + gather-diff. Falls back to
    per-class BLAS if any column is non-contiguous or empty."""
    nz = pi > 0.5                                    # [C, T, TP]
    cnt = nz.sum(axis=1)                             # [C, TP]
    tidx = np.arange(T, dtype=np.int64)[None, :, None]
    t1 = np.where(nz, tidx, -1).max(axis=1)          # [C, TP]
    t0 = np.where(nz, tidx, T).min(axis=1)           # [C, TP]
    if (cnt > 0).all() and (cnt == t1 - t0 + 1).all():
        SX = np.zeros((NX, T + 1, D), dtype=np.float32)
        np.cumsum(X, axis=1, out=SX[:, 1:, :])
        ar = np.arange(NX)[:, None]
        return SX[ar, t1[classe] + 1, :] - SX[ar, t0[classe], :]
    XP = np.empty((NX, TP, D), dtype=np.float32)
    for c in range(C):
        m = classe == c
        if m.any():
            # [n,D,T] @ [T,TP] -> [n,D,TP] -> [n,TP,D]
            XP[m] = np.matmul(X[m].transpose(0, 2, 1), pi[c]).transpose(0, 2, 1)
    return XP


def _sw_weights(xp8_rows, pq):
    """SWI weight layout for one (i-group, p-quarter): [D, QPAIR, NIB, 256].

    Per (p-pair, i-block) the 256-byte weight stream is
    [A(127), B(127), ..., A(0), B(0)] — pairs interleaved, columns
    reversed (A = even p of the pair, B = odd)."""
    arr = xp8_rows[:, pq * QP:(pq + 1) * QP, :].transpose(2, 1, 0)
    v = arr.reshape(D, QPAIR, 2, NIB, 128)           # [d, pp, e, ib, m]
    sw = v[:, :, :, :, ::-1].transpose(0, 1, 3, 4, 2)
    return np.ascontiguousarray(sw).reshape(D, QPAIR, NIB, 256)


def kernel(X, Y, pi, classe):
    global LAST_RUN
    assert X.shape == (NX, T, D) and Y.shape == (NY, TP, D)
    assert pi.shape == (C, T, TP) and classe.shape == (NX,)
    X = np.asarray(X, dtype=np.float32)
    Y = np.asarray(Y, dtype=np.float32)
    pi = np.asarray(pi, dtype=np.float32)
    classe = np.asarray(classe)

    nc = _build_program()

    XP8 = _host_xp(X, pi, classe).astype(F8)         # [NX, TP, D]
    Y8 = Y.astype(F8)                                # [NY, TP, D]
    yts = [np.ascontiguousarray(
        Y8[:, b * QP:(b + 1) * QP, :].transpose(2, 1, 0)) for b in range(GB)]
    in_maps = []
    for k in range(N_CORES):
        a, b = k // GB, k % GB
        in_maps.append(
            {"xpt": _sw_weights(XP8[a * NLI:(a + 1) * NLI], b), "yt": yts[b]})

    trace = bool(os.environ.get("BASS_TRACE"))
    LAST_RUN = run_bass_kernel_spmd(nc, in_maps, list(range(N_CORES)),
                                    trace=trace)
    C3 = np.empty((NX, NY), dtype=np.float32)
    for a in range(GA):
        acc = LAST_RUN.results[a * GB]["c3"].astype(np.float32)
        for b in range(1, GB):
            acc += LAST_RUN.results[a * GB + b]["c3"]
        C3[a * NLI:(a + 1) * NLI] = acc

    # Host epilogue: rank-1 corrections (0.15% of FLOPs).
    row_c = pi.sum(-1)                                 # [C, T]
    col_c = pi.sum(1)                                  # [C, TP]
    SX = np.einsum("itd,itd->it", X, X)                # [NX, T]
    SY = np.einsum("jpd,jpd->jp", Y, Y)                # [NY, TP]
    C1 = np.einsum("it,it->i", SX, row_c[classe])      # [NX]
    C2 = col_c @ SY.T                                  # [C, NY]
    return (C1[:, None] + C2[classe] - 2.0 * C3).astype(np.float32)
